# revision 18
# baseline (speedup 1.0000x reference)
"""Trainium2 kernel for nn_CoxSGDLossFn (topk_masking).

Math (see reference): pair[i,j] = (length[j] > length[i]) * event[i];
p = pair * (1 + rand); thr_i = 3rd-largest of p-row; keep entries p > thr
(at most 2 per row). valid_i = any kept; diagonal of pair set to valid.
row_max_i = max(y_pred) - y_pred[i] (unmasked). Scalar output =
  sum_i valid*(row_max_i + log(sum_j pair_ij exp(y_pred_j - gmax)))
  + 0.05 * sum_{kept (i,j)} |y_pred_j| + 0.05 * sum_i valid_i |y_pred_i|.

Strategy: the only O(n^2) work is finding each row's top-3 eligible entries.
The device streams the 256MB rand matrix once (rows sharded over 8 cores) and,
per row, returns the top-8 COLUMN SEGMENTS (of width 128) ranked by the
segment-max of s = (length[j] > Lh[i]) + rand[i,j]  (Lh = length if event
else 2).  Eligible entries hold s = 1+rand >= 1; everything else < 1, so the
top-3 eligible values always live in the top-3 segments.  The host gathers 4
candidate segments per row (O(n) data) and recomputes the reference math
exactly (f32 p = pair*(1+rand), 3rd-largest threshold, strict >), so device
tie-handling is irrelevant — the device only steers the gather.

Per 128-row tile on each core (DVE only):
  DMA  : rand tile HBM -> SBUF
  DVE  : s = (length_bcast > Lh_row) + rand     (scalar_tensor_tensor)
  DVE  : smax[128, 64] = segmented reduce_max over 128-wide column segments
  DVE  : vals8 = max(smax); seg8 = max_index(vals8, smax)
"""

import numpy as np

N = 8192
NCORES = 8
ROWS_PER_CORE = N // NCORES        # 1024
P = 128
TILES_PER_CORE = ROWS_PER_CORE // P  # 8
SEG = 128                            # column segment width
NSEG = N // SEG                      # 64
CAND = 4                             # candidate segments gathered per row
REG_W = 0.05

# tiles whose combine runs on ACT+GPSIMD instead of DVE
GP_TILES = (1, 3, 5, 7)

_CACHE = {}


def build_bass(repeat=1):
    import concourse.bacc as bacc
    import concourse.mybir as mybir
    from concourse.tile import TileContext

    nc = bacc.Bacc(None, target_bir_lowering=False)
    f32 = mybir.dt.float32
    rand = nc.declare_dram_parameter(
        "rand_shard", [ROWS_PER_CORE, N], f32, isOutput=False
    )
    length_in = nc.declare_dram_parameter("length_full", [N], f32, isOutput=False)
    # lh[p, t] = Lh[row t*128+p of shard]; neg_lh holds -Lh for the ACT path
    lh_in = nc.declare_dram_parameter("lh", [P, TILES_PER_CORE], f32, isOutput=False)
    neg_lh_in = nc.declare_dram_parameter(
        "neg_lh", [P, TILES_PER_CORE], f32, isOutput=False
    )
    out_vals = nc.declare_dram_parameter(
        "top_vals", [ROWS_PER_CORE, 8], f32, isOutput=True
    )
    out_seg = nc.declare_dram_parameter(
        "top_seg", [ROWS_PER_CORE, 8], mybir.dt.uint32, isOutput=True
    )

    with TileContext(nc) as tc:
        with (
            tc.tile_pool(name="const", bufs=1) as cpool,
            tc.tile_pool(name="work", bufs=3) as wpool,
            tc.tile_pool(name="sgn", bufs=2) as gpool,
            tc.tile_pool(name="small", bufs=4) as spool,
        ):
            length_b = cpool.tile([P, N], f32)
            nc.sync.dma_start(
                out=length_b[:], in_=length_in[None, :].broadcast_to((P, N))
            )
            lh_sb = cpool.tile([P, TILES_PER_CORE], f32)
            nc.sync.dma_start(out=lh_sb[:], in_=lh_in[:, :])
            neg_lh_sb = cpool.tile([P, TILES_PER_CORE], f32)
            nc.sync.dma_start(out=neg_lh_sb[:], in_=neg_lh_in[:, :])

            for t in [t for _ in range(repeat) for t in range(TILES_PER_CORE)]:
                s_tile = wpool.tile([P, N], f32, tag="s")
                nc.sync.dma_start(out=s_tile[:], in_=rand[t * P : (t + 1) * P, :])
                if t in GP_TILES:
                    # combine on ACT (sign) + GPSIMD (add), freeing DVE
                    g_tile = gpool.tile([P, N], f32, tag="g")
                    nc.scalar.activation(
                        g_tile[:],
                        length_b[:],
                        mybir.ActivationFunctionType.Sign,
                        bias=neg_lh_sb[:, t : t + 1],
                        scale=1.0,
                    )
                    nc.gpsimd.tensor_tensor(
                        s_tile[:], s_tile[:], g_tile[:], mybir.AluOpType.add
                    )
                else:
                    # s = (length[j] > Lh[i]) + rand, one DVE op, in place
                    nc.vector.scalar_tensor_tensor(
                        s_tile[:],
                        length_b[:],
                        lh_sb[:, t : t + 1],
                        s_tile[:],
                        mybir.AluOpType.is_gt,
                        mybir.AluOpType.add,
                    )
                smax = spool.tile([P, NSEG], f32, tag="smax")
                nc.vector.reduce_max(
                    smax[:],
                    s_tile[:].rearrange("p (g k) -> p g k", k=SEG),
                    axis=mybir.AxisListType.X,
                )
                vals = spool.tile([P, 8], f32, tag="vals")
                seg = spool.tile([P, 8], mybir.dt.uint32, tag="seg")
                nc.vector.max(vals[:], smax[:])
                nc.vector.max_index(seg[:], vals[:], smax[:])
                nc.sync.dma_start(out=out_vals[t * P : (t + 1) * P, :], in_=vals[:])
                nc.sync.dma_start(out=out_seg[t * P : (t + 1) * P, :], in_=seg[:])
    nc.finalize()
    return nc


def run_device(y_pred, length, event, rand_mat, trace=False):
    """Run the bass kernel on 8 cores. Returns (vals[N,8], seg[N,8], results)."""
    from concourse.bass_utils import run_bass_kernel_spmd

    length = np.ascontiguousarray(np.asarray(length, dtype=np.float32))
    event = np.asarray(event, dtype=np.float32)
    rand_mat = np.asarray(rand_mat, dtype=np.float32)

    lh = np.where(event > 0, length, np.float32(2.0)).astype(np.float32)
    lh_pt = lh.reshape(NCORES, TILES_PER_CORE, P).transpose(0, 2, 1)

    if "nc" not in _CACHE:
        _CACHE["nc"] = build_bass()
    nc = _CACHE["nc"]

    in_maps = [
        {
            "rand_shard": np.ascontiguousarray(
                rand_mat[c * ROWS_PER_CORE : (c + 1) * ROWS_PER_CORE]
            ),
            "length_full": length,
            "lh": np.ascontiguousarray(lh_pt[c]),
            "neg_lh": np.ascontiguousarray(-lh_pt[c]),
        }
        for c in range(NCORES)
    ]
    res = run_bass_kernel_spmd(nc, in_maps, list(range(NCORES)), trace=trace)
    vals = np.concatenate([r["top_vals"] for r in res.results], axis=0)
    seg = np.concatenate([r["top_seg"] for r in res.results], axis=0)
    return vals, seg, res


def finish_host(y_pred, length, event, rand_mat, seg8):
    """Exact reference math restricted to CAND candidate segments per row."""
    y32 = np.asarray(y_pred, dtype=np.float32)
    length = np.asarray(length, dtype=np.float32)
    event = np.asarray(event, dtype=np.float32)
    rand_mat = np.asarray(rand_mat, dtype=np.float32)

    segs = np.clip(seg8[:, :CAND].astype(np.int64), 0, NSEG - 1)  # [N, CAND]
    # guard against duplicated segment ids (possible if hw max_index handles
    # duplicate values differently than the sim): mask repeat occurrences
    dup = np.zeros_like(segs, dtype=bool)
    for k in range(1, CAND):
        for j in range(k):
            dup[:, k] |= segs[:, k] == segs[:, j]
    # column indices of the gathered entries: [N, CAND*SEG]
    cols = (segs[:, :, None] * SEG + np.arange(SEG)[None, None, :]).reshape(N, -1)
    rows = np.arange(N)[:, None]
    rand_c = rand_mat[rows, cols]                       # [N, CAND*SEG] f32
    lh = np.where(event > 0, length, np.float32(2.0)).astype(np.float32)
    elig = length[cols] > lh[:, None]                   # strict >, f32-exact
    elig &= ~np.repeat(dup, SEG, axis=1)                # drop duplicated segments
    # p exactly as the reference computes it (f32 1+rand, zeros elsewhere)
    p = np.where(elig, (np.float32(1.0) + rand_c).astype(np.float32), np.float32(0.0))
    # thr = 3rd largest of the full row of p; rows have >= CAND*SEG >= 3 entries
    # and every entry of p outside the candidates is <= the 3rd largest here
    # whenever it could matter (top-3 eligible values live in the candidates;
    # remaining entries are 0 and the candidate set always has >= 3 zeros or
    # eligible values matching the full-row order statistics).
    part = np.partition(p, p.shape[1] - 3, axis=1)
    thr = part[:, -3]                                   # f32 [N]
    keep = p > thr[:, None]                             # <= 2 True per row
    cnt = keep.sum(axis=1)
    valid = cnt > 0

    gmax = np.float32(y32.max())
    y = y32.astype(np.float64)
    e = np.exp(y - np.float64(gmax))
    a = np.abs(y)

    e_cols = e[cols]                                    # [N, CAND*SEG] f64
    a_cols = a[cols]
    se = (keep * e_cols).sum(axis=1) + valid * e
    safe = np.where(valid, se, 1.0)
    row_max = np.float64(gmax) - y
    loss = np.sum(np.where(valid, row_max + np.log(safe), 0.0))
    reg = np.sum(keep * a_cols) + np.sum(valid * a)
    return np.float32(loss + REG_W * reg)


def kernel(y_pred, length, event, rand_mat):
    vals, seg8, _ = run_device(y_pred, length, event, rand_mat, trace=False)
    return finish_host(y_pred, length, event, rand_mat, seg8)
